# revision 4
# baseline (speedup 1.0000x reference)
"""GQA attention (B=2,S=2048,H=2048, 32 Q heads / 8 KV heads, head_dim 64, RoPE,
full non-causal softmax) on 8 TRN2 NeuronCores.

Sharding: tensor-parallel over KV heads. Core i owns KV head i and Q heads
4i..4i+3 (the GQA group). Each core computes its 4 heads of attention plus the
partial O-projection over its 256 output dims; the 8 partials are summed on the
host (pure unshard of the partial-sum shards).

Device layouts are transposed ("dims on partitions") so every matmul contracts
on the partition axis with zero on-device transposes of activations:
  x.T [2048, 4096]        (host-pretransposed, bf16)
  Q.T [256, 4096] = wqT.T-chunks @ x.T-chunks      (2 SBUF tiles of [128, T])
  K.T/V.T [64, 4096]      (col-tiled into one PSUM bank pair)
  scores.T [keys, q] = K.T-slice.T @ Q.T-slice     (per 128-key tile)
  p.T = exp(scores.T/8)   (ScalarE, PSUM->SBUF bf16, scale fused)
  attn_o.T += V-chunk.T @ p.T-chunk ; rowsums += ones.T @ p.T-chunk
  out[t,:] partial = attn_o.T-chunks.T @ woT-chunks
RoPE rotate_half is a partition swap done with tiny SBUF->SBUF DMAs; the sign
flip is folded into the host-precomputed sin table.
"""

import sys

sys.path.insert(0, "/opt/trn_rl_repo")

import math

import ml_dtypes
import numpy as np

import concourse.bass as bass
import concourse.mybir as mybir
import concourse.tile as tile
from concourse import bacc
from concourse.bass_utils import run_bass_kernel_spmd
from concourse.masks import make_identity

BF16 = mybir.dt.bfloat16
F32 = mybir.dt.float32

HIDDEN = 2048
N_HEADS = 32
N_KV_HEADS = 8
HEAD_DIM = 64
KV_GROUPS = 4
ROPE_THETA = 10000.0
BSZ, SEQ = 2, 2048
T = BSZ * SEQ  # 4096 tokens
HCH = HIDDEN // 128  # 16 hidden chunks
NB = T // 512  # 8 token blocks for projections
KT = SEQ // 128  # 16 key tiles per batch
QBLKS = SEQ // 512  # 4 q blocks of 512 per batch
NCORES = 8


def build_nc(trace_scopes: bool = False):
    nc = bacc.Bacc(None, target_bir_lowering=False, debug=False)

    xT = nc.declare_dram_parameter("xT", [HIDDEN, T], BF16, isOutput=False)
    wq = nc.declare_dram_parameter("wq", [HIDDEN, 256], BF16, isOutput=False)
    wkv = nc.declare_dram_parameter("wkv", [HIDDEN, 128], BF16, isOutput=False)
    wo = nc.declare_dram_parameter("wo", [256, HIDDEN], BF16, isOutput=False)
    cosT = nc.declare_dram_parameter("cosT", [128, T], BF16, isOutput=False)
    sinT = nc.declare_dram_parameter("sinT", [128, T], BF16, isOutput=False)
    out = nc.declare_dram_parameter("out", [T, HIDDEN], BF16, isOutput=True)

    with tile.TileContext(nc) as tc:
        _build_body(nc, tc, xT, wq, wkv, wo, cosT, sinT, out)
    nc.compile()
    return nc


def _build_body(nc, tc, xT, wq, wkv, wo, cosT, sinT, out):
    with tc.tile_pool(name="persist", bufs=1) as persist:
        _build_inner(nc, tc, persist, xT, wq, wkv, wo, cosT, sinT, out)


def _build_inner(nc, tc, persist, xT, wq, wkv, wo, cosT, sinT, out):
    # ---------------- persistent SBUF tensors ----------------
    wq_s = persist.tile([128, HCH, 256], BF16, name="wq_s")
    wkv_s = persist.tile([128, HCH, 128], BF16, name="wkv_s")
    wo_s0 = persist.tile([128, HIDDEN], BF16, name="wo_s0")
    wo_s1 = persist.tile([128, HIDDEN], BF16, name="wo_s1")
    cos_s = persist.tile([128, T], BF16, name="cos_s")
    sin_s = persist.tile([128, T], BF16, name="sin_s")
    qt0 = persist.tile([128, T], BF16, name="qt0")  # heads 0(p0-63),1(p64-127)
    qt1 = persist.tile([128, T], BF16, name="qt1")  # heads 2,3
    kvt = persist.tile([128, T], BF16, name="kvt")  # K.T rows 0-63, V.T rows 64-127
    k2t = persist.tile([128, T], BF16, name="k2t")  # K.T duplicated on both halves
    v_s = persist.tile([128, 2 * KT, 64], BF16, name="v_s")  # V chunks [128 keys, 64]
    ones_s = persist.tile([128, 64], BF16, name="ones_s")
    ident = persist.tile([64, 64], BF16, name="ident")
    ao0 = persist.tile([128, T], BF16, name="ao0")  # attn_o.T heads 0,1
    ao1 = persist.tile([128, T], BF16, name="ao1")  # attn_o.T heads 2,3

    nc.gpsimd.memset(ones_s[:], 1.0)
    make_identity(nc, ident[:])

    # weight loads
    for c in range(HCH):
        nc.sync.dma_start(wq_s[:, c, :], wq[c * 128 : (c + 1) * 128, :])
        nc.sync.dma_start(wkv_s[:, c, :], wkv[c * 128 : (c + 1) * 128, :])
    nc.sync.dma_start(wo_s0[:], wo[0:128, :])
    nc.sync.dma_start(wo_s1[:], wo[128:256, :])
    nc.sync.dma_start(cos_s[:], cosT[:])
    nc.sync.dma_start(sin_s[:], sinT[:])

    # ---------------- phase B: QKV projections ----------------
    with (
        tc.tile_pool(name="xs_pool", bufs=4) as xs_pool,
        tc.tile_pool(name="proj_psum", bufs=2, space="PSUM") as proj_psum,
    ):
        for nb in range(NB):
            ts0 = nb * 512
            psq0 = proj_psum.tile([128, 512], F32, name="psq0")
            psq1 = proj_psum.tile([128, 512], F32, name="psq1")
            pskv = proj_psum.tile([128, 512], F32, name="pskv")
            for c in range(HCH):
                xs = xs_pool.tile([128, 512], BF16, name="xs")
                nc.sync.dma_start(xs[:], xT[c * 128 : (c + 1) * 128, ts0 : ts0 + 512])
                st = dict(start=(c == 0), stop=(c == HCH - 1))
                nc.tensor.matmul(psq0[:], wq_s[:, c, 0:128], xs[:], **st)
                nc.tensor.matmul(psq1[:], wq_s[:, c, 128:256], xs[:], **st)
                nc.tensor.matmul(pskv[0:64, :], wkv_s[:, c, 0:64], xs[:], **st)
                nc.tensor.matmul(pskv[64:128, :], wkv_s[:, c, 64:128], xs[:], **st)
            nc.vector.tensor_copy(qt0[:, ts0 : ts0 + 512], psq0[:])
            nc.vector.tensor_copy(qt1[:, ts0 : ts0 + 512], psq1[:])
            nc.vector.tensor_copy(kvt[:, ts0 : ts0 + 512], pskv[:])

    # ---------------- phase C: RoPE + K duplication + V transpose ------------
    with (
        tc.tile_pool(name="rope_pool", bufs=1) as rope_pool,
        tc.tile_pool(name="tr_psum", bufs=2, space="PSUM") as tr_psum,
        tc.tile_pool(name="vtmp_pool", bufs=1) as vtmp_pool,
    ):
        for qt in (qt0, qt1):
            sq = rope_pool.tile([128, T], BF16, name="sq")
            nc.sync.dma_start(sq[0:32, :], qt[32:64, :])
            nc.sync.dma_start(sq[32:64, :], qt[0:32, :])
            nc.sync.dma_start(sq[64:96, :], qt[96:128, :])
            nc.sync.dma_start(sq[96:128, :], qt[64:96, :])
            nc.vector.tensor_tensor(sq[:], sq[:], sin_s[:], mybir.AluOpType.mult)
            nc.vector.tensor_tensor(qt[:], qt[:], cos_s[:], mybir.AluOpType.mult)
            nc.vector.tensor_tensor(qt[:], qt[:], sq[:], mybir.AluOpType.add)
        # K rows 0-63 of kvt
        sk = rope_pool.tile([64, T], BF16, name="sk")
        nc.sync.dma_start(sk[0:32, :], kvt[32:64, :])
        nc.sync.dma_start(sk[32:64, :], kvt[0:32, :])
        nc.vector.tensor_tensor(sk[:], sk[:], sin_s[0:64, :], mybir.AluOpType.mult)
        nc.vector.tensor_tensor(
            kvt[0:64, :], kvt[0:64, :], cos_s[0:64, :], mybir.AluOpType.mult
        )
        nc.vector.tensor_tensor(kvt[0:64, :], kvt[0:64, :], sk[:], mybir.AluOpType.add)
        # duplicate K.T to both halves of k2t
        nc.sync.dma_start(k2t[0:64, :], kvt[0:64, :])
        nc.sync.dma_start(k2t[64:128, :], kvt[0:64, :])
        # V: move V.T rows 64-127 down to 0-63, then PE-transpose chunks
        vtmp = vtmp_pool.tile([64, T], BF16, name="vtmp")
        nc.sync.dma_start(vtmp[:], kvt[64:128, :])
        for c in range(2 * KT):
            pst = tr_psum.tile([128, 64], BF16, name="pst")
            nc.tensor.transpose(pst[:], vtmp[:, c * 128 : (c + 1) * 128], ident[:])
            nc.vector.tensor_copy(v_s[:, c, :], pst[:])

    # ---------------- phase D: attention + O-projection ----------------
    with (
        tc.tile_pool(name="sc_psum", bufs=2, space="PSUM") as sc_psum,
        tc.tile_pool(name="av_psum", bufs=1, space="PSUM") as av_psum,
        tc.tile_pool(name="rs_psum", bufs=1, space="PSUM") as rs_psum,
        tc.tile_pool(name="op_psum", bufs=2, space="PSUM") as op_psum,
        tc.tile_pool(name="pt_pool", bufs=3) as pt_pool,
        tc.tile_pool(name="rrs_pool", bufs=2) as rrs_pool,
        tc.tile_pool(name="ost_pool", bufs=3) as ost_pool,
    ):
        for b in range(BSZ):
            for qb in range(QBLKS):
                qs = b * SEQ + qb * 512
                for hp, (qt, ao) in enumerate(((qt0, ao0), (qt1, ao1))):
                    pv = av_psum.tile([128, 512], F32, name="pv")
                    po = rs_psum.tile([128, 512], F32, name="po")
                    for kt in range(KT):
                        kr = b * SEQ + kt * 128
                        kc = b * KT + kt
                        psa = sc_psum.tile([128, 1024], F32, name="psa")
                        # scores.T for even head (lanes 0-63) and odd (64-127)
                        nc.tensor.matmul(
                            psa[:, 0:512],
                            k2t[0:64, kr : kr + 128],
                            qt[0:64, qs : qs + 512],
                        )
                        nc.tensor.matmul(
                            psa[:, 512:1024],
                            k2t[64:128, kr : kr + 128],
                            qt[64:128, qs : qs + 512],
                        )
                        pt = pt_pool.tile([128, 1024], BF16, name="pt")
                        nc.scalar.activation(
                            pt[:],
                            psa[:],
                            mybir.ActivationFunctionType.Exp,
                            scale=1.0 / math.sqrt(HEAD_DIM),
                        )
                        st = dict(start=(kt == 0), stop=(kt == KT - 1))
                        nc.tensor.matmul(
                            pv[0:64, :], v_s[:, kc, :], pt[:, 0:512], **st
                        )
                        nc.tensor.matmul(
                            pv[64:128, :], v_s[:, kc, :], pt[:, 512:1024], **st
                        )
                        nc.tensor.matmul(po[0:64, :], ones_s[:], pt[:, 0:512], **st)
                        nc.tensor.matmul(
                            po[64:128, :], ones_s[:], pt[:, 512:1024], **st
                        )
                    rrs = rrs_pool.tile([128, 512], F32, name="rrs")
                    nc.vector.reciprocal(rrs[:], po[:])
                    nc.vector.tensor_tensor(
                        ao[:, qs : qs + 512], pv[:], rrs[:], mybir.AluOpType.mult
                    )
                # O-projection for these 512 tokens
                for tb in range(4):
                    ts0 = qs + tb * 128
                    for oj in range(4):
                        pop = op_psum.tile([128, 512], F32, name="pop")
                        nc.tensor.matmul(
                            pop[:],
                            ao0[:, ts0 : ts0 + 128],
                            wo_s0[:, oj * 512 : (oj + 1) * 512],
                            start=True,
                            stop=False,
                        )
                        nc.tensor.matmul(
                            pop[:],
                            ao1[:, ts0 : ts0 + 128],
                            wo_s1[:, oj * 512 : (oj + 1) * 512],
                            start=False,
                            stop=True,
                        )
                        ost = ost_pool.tile([128, 512], BF16, name="ost")
                        nc.vector.tensor_copy(ost[:], pop[:])
                        nc.sync.dma_start(
                            out[ts0 : ts0 + 128, oj * 512 : (oj + 1) * 512], ost[:]
                        )


def _host_prep(hidden_states, position_ids, Wq, Wk, Wv, Wo):
    bf = ml_dtypes.bfloat16
    x = np.ascontiguousarray(hidden_states.reshape(T, HIDDEN))
    xT = np.ascontiguousarray(x.T).astype(bf)

    # RoPE tables, transposed to [64, T], sign of sin folded for rotate_half,
    # then stacked twice to cover two heads per SBUF tile.
    inv_freq = 1.0 / (
        ROPE_THETA ** (np.arange(0, HEAD_DIM, 2, dtype=np.float32) / HEAD_DIM)
    )
    pos = position_ids.astype(np.float32).reshape(BSZ, SEQ)
    freqs = pos[:, :, None] * inv_freq[None, None, :]  # [B, S, 32]
    emb = np.concatenate([freqs, freqs], axis=-1)  # [B, S, 64]
    cos = np.cos(emb).reshape(T, HEAD_DIM).T  # [64, T]
    sin = np.sin(emb).reshape(T, HEAD_DIM).T.copy()
    sin[0:32, :] *= -1.0  # rotate_half sign fold
    cosT = np.ascontiguousarray(np.concatenate([cos, cos], axis=0)).astype(bf)
    sinT = np.ascontiguousarray(np.concatenate([sin, sin], axis=0)).astype(bf)

    in_maps = []
    for c in range(NCORES):
        q0 = c * KV_GROUPS * HEAD_DIM  # 256*c
        wq_c = np.ascontiguousarray(Wq[q0 : q0 + 256, :].T).astype(bf)  # [2048, 256]
        wk_c = Wk[c * 64 : (c + 1) * 64, :].T  # [2048, 64]
        wv_c = Wv[c * 64 : (c + 1) * 64, :].T
        wkv_c = np.ascontiguousarray(np.concatenate([wk_c, wv_c], axis=1)).astype(bf)
        wo_c = np.ascontiguousarray(Wo[:, q0 : q0 + 256].T).astype(bf)  # [256, 2048]
        in_maps.append(
            {
                "xT": xT,
                "wq": wq_c,
                "wkv": wkv_c,
                "wo": wo_c,
                "cosT": cosT,
                "sinT": sinT,
            }
        )
    return in_maps


_RUN_KW = {}


def kernel(hidden_states, position_ids, Wq, Wk, Wv, Wo):
    in_maps = _host_prep(hidden_states, position_ids, Wq, Wk, Wv, Wo)
    nc = build_nc()
    res = run_bass_kernel_spmd(nc, in_maps, core_ids=list(range(NCORES)), **_RUN_KW)
    acc = np.zeros((T, HIDDEN), dtype=np.float32)
    for i in range(NCORES):
        acc += res.results[i]["out"].astype(np.float32)
    if _RUN_KW.get("trace"):
        kernel.last_exec_time_ns = res.exec_time_ns
        kernel.last_result = res
    return acc.reshape(BSZ, SEQ, HIDDEN)


# revision 6
# speedup vs baseline: 1.1454x; 1.1454x over previous
"""GQA attention (B=2,S=2048,H=2048, 32 Q heads / 8 KV heads, head_dim 64, RoPE,
full non-causal softmax) on 8 TRN2 NeuronCores.

Sharding: tensor-parallel over KV heads. Core i owns KV head i and Q heads
4i..4i+3 (the GQA group). Each core computes its 4 heads of attention plus the
partial O-projection over its 256 output dims; the 8 partials are summed on the
host (pure unshard of the partial-sum shards).

Device layouts are transposed ("dims on partitions") so every matmul contracts
on the partition axis with zero on-device transposes of activations:
  x.T [2048, 4096]        (host-pretransposed, bf16)
  Q.T [256, 4096] = wqT.T-chunks @ x.T-chunks      (2 SBUF tiles of [128, T])
  K.T/V.T [64, 4096]      (col-tiled into one PSUM bank pair)
  scores.T [keys, q] = K.T-slice.T @ Q.T-slice     (per 128-key tile)
  p.T = exp(scores.T/8)   (ScalarE, PSUM->SBUF bf16, scale fused)
  attn_o.T += V-chunk.T @ p.T-chunk ; rowsums += ones.T @ p.T-chunk
  out[t,:] partial = attn_o.T-chunks.T @ woT-chunks
RoPE rotate_half is a partition swap done with tiny SBUF->SBUF DMAs; the sign
flip is folded into the host-precomputed sin table.
"""

import sys

sys.path.insert(0, "/opt/trn_rl_repo")

import math

import ml_dtypes
import numpy as np

import concourse.bass as bass
import concourse.mybir as mybir
import concourse.tile as tile
from concourse import bacc
from concourse.bass_utils import run_bass_kernel_spmd
from concourse.masks import make_identity

BF16 = mybir.dt.bfloat16
F32 = mybir.dt.float32

HIDDEN = 2048
N_HEADS = 32
N_KV_HEADS = 8
HEAD_DIM = 64
KV_GROUPS = 4
ROPE_THETA = 10000.0
BSZ, SEQ = 2, 2048
T = BSZ * SEQ  # 4096 tokens
HCH = HIDDEN // 128  # 16 hidden chunks
NB = T // 512  # 8 token blocks for projections
KT = SEQ // 128  # 16 key tiles per batch
QBLKS = SEQ // 512  # 4 q blocks of 512 per batch
NCORES = 8


def build_nc(trace_scopes: bool = False):
    nc = bacc.Bacc(None, target_bir_lowering=False, debug=False)

    xT = nc.declare_dram_parameter("xT", [NB, HCH, 128, 512], BF16, isOutput=False)
    wq = nc.declare_dram_parameter("wq", [HIDDEN, 256], BF16, isOutput=False)
    wkv = nc.declare_dram_parameter("wkv", [HIDDEN, 128], BF16, isOutput=False)
    wo = nc.declare_dram_parameter("wo", [256, HIDDEN], BF16, isOutput=False)
    cosT = nc.declare_dram_parameter("cosT", [128, T], BF16, isOutput=False)
    sinT = nc.declare_dram_parameter("sinT", [128, T], BF16, isOutput=False)
    out = nc.declare_dram_parameter("out", [T, HIDDEN], BF16, isOutput=True)

    with tile.TileContext(nc) as tc:
        _build_body(nc, tc, xT, wq, wkv, wo, cosT, sinT, out)
    nc.compile()
    return nc


def _build_body(nc, tc, xT, wq, wkv, wo, cosT, sinT, out):
    with tc.tile_pool(name="persist", bufs=1) as persist:
        _build_inner(nc, tc, persist, xT, wq, wkv, wo, cosT, sinT, out)


def _build_inner(nc, tc, persist, xT, wq, wkv, wo, cosT, sinT, out):
    # ---------------- persistent SBUF tensors ----------------
    wq_s = persist.tile([128, HCH, 256], BF16, name="wq_s")
    wkv_s = persist.tile([128, HCH, 128], BF16, name="wkv_s")
    wo_s0 = persist.tile([128, HIDDEN], BF16, name="wo_s0")
    wo_s1 = persist.tile([128, HIDDEN], BF16, name="wo_s1")
    cos_s = persist.tile([128, T], BF16, name="cos_s")
    sin_s = persist.tile([128, T], BF16, name="sin_s")
    qt0 = persist.tile([128, T], BF16, name="qt0")  # heads 0(p0-63),1(p64-127)
    qt1 = persist.tile([128, T], BF16, name="qt1")  # heads 2,3
    kvt = persist.tile([128, T], BF16, name="kvt")  # K.T rows 0-63, V.T rows 64-127
    k2t = persist.tile([128, T], BF16, name="k2t")  # K.T duplicated on both halves
    # [ones | V | ones]: even-head lhsT = [:,kc,64:192] = [V|ones],
    # odd-head lhsT = [:,kc,0:128] = [ones|V]
    v_s = persist.tile([128, 2 * KT, 192], BF16, name="v_s")
    ident = persist.tile([64, 64], BF16, name="ident")
    ao0 = persist.tile([128, T], BF16, name="ao0")  # attn_o.T heads 0,1
    ao1 = persist.tile([128, T], BF16, name="ao1")  # attn_o.T heads 2,3

    nc.gpsimd.memset(v_s.rearrange("p c m -> p (c m)")[:, :], 1.0)
    make_identity(nc, ident[:])

    # weight loads
    for c in range(HCH):
        nc.sync.dma_start(wq_s[:, c, :], wq[c * 128 : (c + 1) * 128, :])
        nc.sync.dma_start(wkv_s[:, c, :], wkv[c * 128 : (c + 1) * 128, :])
    nc.sync.dma_start(wo_s0[:], wo[0:128, :])
    nc.sync.dma_start(wo_s1[:], wo[128:256, :])
    nc.sync.dma_start(cos_s[:], cosT[:])
    nc.sync.dma_start(sin_s[:], sinT[:])

    # ---------------- phase B: QKV projections ----------------
    with (
        tc.tile_pool(name="xs_pool", bufs=4) as xs_pool,
        tc.tile_pool(name="proj_psum", bufs=2, space="PSUM") as proj_psum,
    ):
        for nb in range(NB):
            ts0 = nb * 512
            psq0 = proj_psum.tile([128, 512], F32, name="psq0")
            psq1 = proj_psum.tile([128, 512], F32, name="psq1")
            pskv = proj_psum.tile([128, 512], F32, name="pskv")
            for c in range(HCH):
                xs = xs_pool.tile([128, 512], BF16, name="xs")
                nc.sync.dma_start(xs[:], xT[nb, c])
                st = dict(start=(c == 0), stop=(c == HCH - 1))
                nc.tensor.matmul(psq0[:], wq_s[:, c, 0:128], xs[:], **st)
                nc.tensor.matmul(psq1[:], wq_s[:, c, 128:256], xs[:], **st)
                nc.tensor.matmul(pskv[0:64, :], wkv_s[:, c, 0:64], xs[:], **st)
                nc.tensor.matmul(pskv[64:128, :], wkv_s[:, c, 64:128], xs[:], **st)
            nc.vector.tensor_copy(qt0[:, ts0 : ts0 + 512], psq0[:])
            nc.vector.tensor_copy(qt1[:, ts0 : ts0 + 512], psq1[:])
            nc.vector.tensor_copy(kvt[:, ts0 : ts0 + 512], pskv[:])

    # ---------------- phase C: RoPE + K duplication + V transpose ------------
    with (
        tc.tile_pool(name="rope_pool", bufs=1) as rope_pool,
        tc.tile_pool(name="tr_psum", bufs=2, space="PSUM") as tr_psum,
        tc.tile_pool(name="vtmp_pool", bufs=1) as vtmp_pool,
    ):
        for qt in (qt0, qt1):
            sq = rope_pool.tile([128, T], BF16, name="sq")
            nc.sync.dma_start(sq[0:32, :], qt[32:64, :])
            nc.sync.dma_start(sq[32:64, :], qt[0:32, :])
            nc.sync.dma_start(sq[64:96, :], qt[96:128, :])
            nc.sync.dma_start(sq[96:128, :], qt[64:96, :])
            nc.vector.tensor_tensor(sq[:], sq[:], sin_s[:], mybir.AluOpType.mult)
            nc.vector.tensor_tensor(qt[:], qt[:], cos_s[:], mybir.AluOpType.mult)
            nc.vector.tensor_tensor(qt[:], qt[:], sq[:], mybir.AluOpType.add)
        # K rows 0-63 of kvt
        sk = rope_pool.tile([64, T], BF16, name="sk")
        nc.sync.dma_start(sk[0:32, :], kvt[32:64, :])
        nc.sync.dma_start(sk[32:64, :], kvt[0:32, :])
        nc.vector.tensor_tensor(sk[:], sk[:], sin_s[0:64, :], mybir.AluOpType.mult)
        nc.vector.tensor_tensor(
            kvt[0:64, :], kvt[0:64, :], cos_s[0:64, :], mybir.AluOpType.mult
        )
        nc.vector.tensor_tensor(kvt[0:64, :], kvt[0:64, :], sk[:], mybir.AluOpType.add)
        # duplicate K.T to both halves of k2t
        nc.sync.dma_start(k2t[0:64, :], kvt[0:64, :])
        nc.sync.dma_start(k2t[64:128, :], kvt[0:64, :])
        # V: move V.T rows 64-127 down to 0-63, then PE-transpose chunks
        vtmp = vtmp_pool.tile([64, T], BF16, name="vtmp")
        nc.sync.dma_start(vtmp[:], kvt[64:128, :])
        for c in range(2 * KT):
            pst = tr_psum.tile([128, 64], BF16, name="pst")
            nc.tensor.transpose(pst[:], vtmp[:, c * 128 : (c + 1) * 128], ident[:])
            nc.vector.tensor_copy(v_s[:, c, 64:128], pst[:])

    # ---------------- phase D: attention + O-projection ----------------
    with (
        tc.tile_pool(name="sc_psum", bufs=2, space="PSUM") as sc_psum,
        tc.tile_pool(name="av_psum", bufs=2, space="PSUM") as av_psum,
        tc.tile_pool(name="pt_pool", bufs=4) as pt_pool,
        tc.tile_pool(name="rrs_pool", bufs=2) as rrs_pool,
        tc.tile_pool(name="ost_pool", bufs=3) as ost_pool,
    ):
        for b in range(BSZ):
            for qb in range(QBLKS):
                qs = b * SEQ + qb * 512
                for hp, (qt, ao) in enumerate(((qt0, ao0), (qt1, ao1))):
                    pse = av_psum.tile([128, 512], F32, name="pse")
                    pso = av_psum.tile([128, 512], F32, name="pso")
                    for kt in range(KT):
                        kr = b * SEQ + kt * 128
                        kc = b * KT + kt
                        psa = sc_psum.tile([128, 1024], F32, name="psa")
                        # scores.T for even head (lanes 0-63) and odd (64-127)
                        nc.tensor.matmul(
                            psa[:, 0:512],
                            k2t[0:64, kr : kr + 128],
                            qt[0:64, qs : qs + 512],
                        )
                        nc.tensor.matmul(
                            psa[:, 512:1024],
                            k2t[64:128, kr : kr + 128],
                            qt[64:128, qs : qs + 512],
                        )
                        pt = pt_pool.tile([128, 1024], BF16, name="pt")
                        nc.scalar.activation(
                            pt[:],
                            psa[:],
                            mybir.ActivationFunctionType.Exp,
                            scale=1.0 / math.sqrt(HEAD_DIM),
                        )
                        st = dict(start=(kt == 0), stop=(kt == KT - 1))
                        # even head: [V|ones] -> rows 0-63 attn, 64-127 rowsums
                        nc.tensor.matmul(
                            pse[:], v_s[:, kc, 64:192], pt[:, 0:512], **st
                        )
                        # odd head: [ones|V] -> rows 0-63 rowsums, 64-127 attn
                        nc.tensor.matmul(
                            pso[:], v_s[:, kc, 0:128], pt[:, 512:1024], **st
                        )
                    # assemble reciprocal rowsums lane-aligned with attn rows
                    rsh = rrs_pool.tile([128, 512], F32, name="rsh")
                    nc.vector.tensor_copy(rsh[64:128, :], pse[64:128, :])
                    nc.vector.tensor_copy(rsh[0:64, :], pso[0:64, :])
                    rrs = rrs_pool.tile([128, 512], F32, name="rrs")
                    nc.sync.dma_start(rrs[0:64, :], rsh[64:128, :])
                    nc.sync.dma_start(rrs[64:128, :], rsh[0:64, :])
                    rri = rrs_pool.tile([128, 512], F32, name="rri")
                    nc.vector.reciprocal_approx_fast(rri[:], rrs[:])
                    nc.vector.tensor_tensor(
                        ao[0:64, qs : qs + 512],
                        pse[0:64, :],
                        rri[0:64, :],
                        mybir.AluOpType.mult,
                    )
                    nc.vector.tensor_tensor(
                        ao[64:128, qs : qs + 512],
                        pso[64:128, :],
                        rri[64:128, :],
                        mybir.AluOpType.mult,
                    )
                # O-projection for these 512 tokens
                for tb in range(4):
                    ts0 = qs + tb * 128
                    for oj in range(4):
                        pop = av_psum.tile([128, 512], F32, name="pop", tag="pse")
                        nc.tensor.matmul(
                            pop[:],
                            ao0[:, ts0 : ts0 + 128],
                            wo_s0[:, oj * 512 : (oj + 1) * 512],
                            start=True,
                            stop=False,
                        )
                        nc.tensor.matmul(
                            pop[:],
                            ao1[:, ts0 : ts0 + 128],
                            wo_s1[:, oj * 512 : (oj + 1) * 512],
                            start=False,
                            stop=True,
                        )
                        ost = ost_pool.tile([128, 512], BF16, name="ost")
                        nc.vector.tensor_copy(ost[:], pop[:])
                        nc.sync.dma_start(
                            out[ts0 : ts0 + 128, oj * 512 : (oj + 1) * 512], ost[:]
                        )


def _host_prep(hidden_states, position_ids, Wq, Wk, Wv, Wo):
    bf = ml_dtypes.bfloat16
    x = np.ascontiguousarray(hidden_states.reshape(T, HIDDEN))
    xT = x.T.astype(bf)  # [HIDDEN, T]
    # block to [NB, HCH, 128, 512] so each projection tile is one contiguous read
    xT = np.ascontiguousarray(
        xT.reshape(HCH, 128, NB, 512).transpose(2, 0, 1, 3)
    )

    # RoPE tables, transposed to [64, T], sign of sin folded for rotate_half,
    # then stacked twice to cover two heads per SBUF tile.
    inv_freq = 1.0 / (
        ROPE_THETA ** (np.arange(0, HEAD_DIM, 2, dtype=np.float32) / HEAD_DIM)
    )
    pos = position_ids.astype(np.float32).reshape(BSZ, SEQ)
    freqs = pos[:, :, None] * inv_freq[None, None, :]  # [B, S, 32]
    emb = np.concatenate([freqs, freqs], axis=-1)  # [B, S, 64]
    cos = np.cos(emb).reshape(T, HEAD_DIM).T  # [64, T]
    sin = np.sin(emb).reshape(T, HEAD_DIM).T.copy()
    sin[0:32, :] *= -1.0  # rotate_half sign fold
    cosT = np.ascontiguousarray(np.concatenate([cos, cos], axis=0)).astype(bf)
    sinT = np.ascontiguousarray(np.concatenate([sin, sin], axis=0)).astype(bf)

    in_maps = []
    for c in range(NCORES):
        q0 = c * KV_GROUPS * HEAD_DIM  # 256*c
        wq_c = np.ascontiguousarray(Wq[q0 : q0 + 256, :].T).astype(bf)  # [2048, 256]
        wk_c = Wk[c * 64 : (c + 1) * 64, :].T  # [2048, 64]
        wv_c = Wv[c * 64 : (c + 1) * 64, :].T
        wkv_c = np.ascontiguousarray(np.concatenate([wk_c, wv_c], axis=1)).astype(bf)
        wo_c = np.ascontiguousarray(Wo[:, q0 : q0 + 256].T).astype(bf)  # [256, 2048]
        in_maps.append(
            {
                "xT": xT,
                "wq": wq_c,
                "wkv": wkv_c,
                "wo": wo_c,
                "cosT": cosT,
                "sinT": sinT,
            }
        )
    return in_maps


_RUN_KW = {}


def kernel(hidden_states, position_ids, Wq, Wk, Wv, Wo):
    in_maps = _host_prep(hidden_states, position_ids, Wq, Wk, Wv, Wo)
    nc = build_nc()
    res = run_bass_kernel_spmd(nc, in_maps, core_ids=list(range(NCORES)), **_RUN_KW)
    acc = np.zeros((T, HIDDEN), dtype=np.float32)
    for i in range(NCORES):
        acc += res.results[i]["out"].astype(np.float32)
    if _RUN_KW.get("trace"):
        kernel.last_exec_time_ns = res.exec_time_ns
        kernel.last_result = res
    return acc.reshape(BSZ, SEQ, HIDDEN)


# revision 8
# speedup vs baseline: 1.1659x; 1.0179x over previous
"""GQA attention (B=2,S=2048,H=2048, 32 Q heads / 8 KV heads, head_dim 64, RoPE,
full non-causal softmax) on 8 TRN2 NeuronCores.

Sharding: tensor-parallel over KV heads. Core i owns KV head i and Q heads
4i..4i+3 (the GQA group). Each core computes its 4 heads of attention plus the
partial O-projection over its 256 output dims; the 8 partials are summed on the
host (pure unshard of the partial-sum shards).

Device layouts are transposed ("dims on partitions") so every matmul contracts
on the partition axis with zero on-device transposes of activations:
  x.T [2048, 4096]        (host-pretransposed, bf16)
  Q.T [256, 4096] = wqT.T-chunks @ x.T-chunks      (2 SBUF tiles of [128, T])
  K.T/V.T [64, 4096]      (col-tiled into one PSUM bank pair)
  scores.T [keys, q] = K.T-slice.T @ Q.T-slice     (per 128-key tile)
  p.T = exp(scores.T/8)   (ScalarE, PSUM->SBUF bf16, scale fused)
  attn_o.T += V-chunk.T @ p.T-chunk ; rowsums += ones.T @ p.T-chunk
  out[t,:] partial = attn_o.T-chunks.T @ woT-chunks
RoPE rotate_half is a partition swap done with tiny SBUF->SBUF DMAs; the sign
flip is folded into the host-precomputed sin table.
"""

import sys

sys.path.insert(0, "/opt/trn_rl_repo")

import math

import ml_dtypes
import numpy as np

import concourse.bass as bass
import concourse.mybir as mybir
import concourse.tile as tile
from concourse import bacc
from concourse.bass_utils import run_bass_kernel_spmd
from concourse.masks import make_identity

BF16 = mybir.dt.bfloat16
F32 = mybir.dt.float32

HIDDEN = 2048
N_HEADS = 32
N_KV_HEADS = 8
HEAD_DIM = 64
KV_GROUPS = 4
ROPE_THETA = 10000.0
BSZ, SEQ = 2, 2048
T = BSZ * SEQ  # 4096 tokens
HCH = HIDDEN // 128  # 16 hidden chunks
NB = T // 512  # 8 token blocks for projections
KT = SEQ // 128  # 16 key tiles per batch
QBLKS = SEQ // 512  # 4 q blocks of 512 per batch
NCORES = 8


def build_nc(trace_scopes: bool = False):
    nc = bacc.Bacc(None, target_bir_lowering=False, debug=False)

    xT = nc.declare_dram_parameter("xT", [NB // 2, HCH, 128, 1024], BF16, isOutput=False)
    wq = nc.declare_dram_parameter("wq", [HIDDEN, 256], BF16, isOutput=False)
    wkv = nc.declare_dram_parameter("wkv", [HIDDEN, 128], BF16, isOutput=False)
    wo = nc.declare_dram_parameter("wo", [256, HIDDEN], BF16, isOutput=False)
    cosT = nc.declare_dram_parameter("cosT", [128, T], BF16, isOutput=False)
    sinT = nc.declare_dram_parameter("sinT", [128, T], BF16, isOutput=False)
    out = nc.declare_dram_parameter("out", [T, HIDDEN], BF16, isOutput=True)

    with tile.TileContext(nc) as tc:
        _build_body(nc, tc, xT, wq, wkv, wo, cosT, sinT, out)
    nc.compile()
    return nc


def _build_body(nc, tc, xT, wq, wkv, wo, cosT, sinT, out):
    with tc.tile_pool(name="persist", bufs=1) as persist:
        _build_inner(nc, tc, persist, xT, wq, wkv, wo, cosT, sinT, out)


def _build_inner(nc, tc, persist, xT, wq, wkv, wo, cosT, sinT, out):
    # ---------------- persistent SBUF tensors ----------------
    wq_s = persist.tile([128, HCH, 256], BF16, name="wq_s")
    wkv_s = persist.tile([128, HCH, 128], BF16, name="wkv_s")
    wo_s0 = persist.tile([128, HIDDEN], BF16, name="wo_s0")
    wo_s1 = persist.tile([128, HIDDEN], BF16, name="wo_s1")
    cos_s = persist.tile([128, T], BF16, name="cos_s")
    sin_s = persist.tile([128, T], BF16, name="sin_s")
    qt0 = persist.tile([128, T], BF16, name="qt0")  # heads 0(p0-63),1(p64-127)
    qt1 = persist.tile([128, T], BF16, name="qt1")  # heads 2,3
    kvt = persist.tile([128, T], BF16, name="kvt")  # K.T rows 0-63, V.T rows 64-127
    k2t = persist.tile([128, T], BF16, name="k2t")  # K.T duplicated on both halves
    # [ones | V | ones]: even-head lhsT = [:,kc,64:192] = [V|ones],
    # odd-head lhsT = [:,kc,0:128] = [ones|V]
    v_s = persist.tile([128, 2 * KT, 192], BF16, name="v_s")
    ident = persist.tile([64, 64], BF16, name="ident")
    ao0 = persist.tile([128, T], BF16, name="ao0")  # attn_o.T heads 0,1
    ao1 = persist.tile([128, T], BF16, name="ao1")  # attn_o.T heads 2,3

    nc.gpsimd.memset(v_s.rearrange("p c m -> p (c m)")[:, :], 1.0)
    make_identity(nc, ident[:])

    # weight loads
    for c in range(HCH):
        nc.sync.dma_start(wq_s[:, c, :], wq[c * 128 : (c + 1) * 128, :])
        nc.sync.dma_start(wkv_s[:, c, :], wkv[c * 128 : (c + 1) * 128, :])
    nc.sync.dma_start(wo_s0[:], wo[0:128, :])
    nc.sync.dma_start(wo_s1[:], wo[128:256, :])
    nc.sync.dma_start(cos_s[:], cosT[:])
    nc.sync.dma_start(sin_s[:], sinT[:])

    # ---------------- phase B: QKV projections ----------------
    with (
        tc.tile_pool(name="xs_pool", bufs=3) as xs_pool,
        tc.tile_pool(name="proj_psum", bufs=1, space="PSUM") as proj_psum,
    ):
        for np_ in range(NB // 2):
            ts0 = np_ * 1024
            psq0 = proj_psum.tile([128, 1024], F32, name="psq0")
            psq1 = proj_psum.tile([128, 1024], F32, name="psq1")
            pskv = proj_psum.tile([128, 1024], F32, name="pskv")
            for c in range(HCH):
                xs = xs_pool.tile([128, 1024], BF16, name="xs")
                nc.sync.dma_start(xs[:], xT[np_, c])
                st = dict(start=(c == 0), stop=(c == HCH - 1))
                for h in range(2):
                    sl = slice(h * 512, (h + 1) * 512)
                    nc.tensor.matmul(psq0[:, sl], wq_s[:, c, 0:128], xs[:, sl], **st)
                    nc.tensor.matmul(psq1[:, sl], wq_s[:, c, 128:256], xs[:, sl], **st)
                    nc.tensor.matmul(pskv[0:64, sl], wkv_s[:, c, 0:64], xs[:, sl], **st)
                    nc.tensor.matmul(pskv[64:128, sl], wkv_s[:, c, 64:128], xs[:, sl], **st)
            nc.vector.tensor_copy(qt0[:, ts0 : ts0 + 1024], psq0[:])
            nc.vector.tensor_copy(qt1[:, ts0 : ts0 + 1024], psq1[:])
            nc.vector.tensor_copy(kvt[:, ts0 : ts0 + 1024], pskv[:])

    # ---------------- phase C: RoPE + K duplication + V transpose ------------
    with (
        tc.tile_pool(name="rope_pool", bufs=1) as rope_pool,
        tc.tile_pool(name="tr_psum", bufs=2, space="PSUM") as tr_psum,
        tc.tile_pool(name="vtmp_pool", bufs=1) as vtmp_pool,
    ):
        for qt in (qt0, qt1):
            sq = rope_pool.tile([128, T], BF16, name="sq")
            nc.sync.dma_start(sq[0:32, :], qt[32:64, :])
            nc.sync.dma_start(sq[32:64, :], qt[0:32, :])
            nc.sync.dma_start(sq[64:96, :], qt[96:128, :])
            nc.sync.dma_start(sq[96:128, :], qt[64:96, :])
            nc.vector.tensor_tensor(sq[:], sq[:], sin_s[:], mybir.AluOpType.mult)
            nc.vector.tensor_tensor(qt[:], qt[:], cos_s[:], mybir.AluOpType.mult)
            nc.vector.tensor_tensor(qt[:], qt[:], sq[:], mybir.AluOpType.add)
        # K rows 0-63 of kvt
        sk = rope_pool.tile([64, T], BF16, name="sk")
        nc.sync.dma_start(sk[0:32, :], kvt[32:64, :])
        nc.sync.dma_start(sk[32:64, :], kvt[0:32, :])
        nc.vector.tensor_tensor(sk[:], sk[:], sin_s[0:64, :], mybir.AluOpType.mult)
        nc.vector.tensor_tensor(
            kvt[0:64, :], kvt[0:64, :], cos_s[0:64, :], mybir.AluOpType.mult
        )
        nc.vector.tensor_tensor(kvt[0:64, :], kvt[0:64, :], sk[:], mybir.AluOpType.add)
        # duplicate K.T to both halves of k2t
        nc.sync.dma_start(k2t[0:64, :], kvt[0:64, :])
        nc.sync.dma_start(k2t[64:128, :], kvt[0:64, :])
        # V: move V.T rows 64-127 down to 0-63, then PE-transpose chunks
        vtmp = vtmp_pool.tile([64, T], BF16, name="vtmp")
        nc.sync.dma_start(vtmp[:], kvt[64:128, :])
        for c in range(2 * KT):
            pst = tr_psum.tile([128, 64], BF16, name="pst")
            nc.tensor.transpose(pst[:], vtmp[:, c * 128 : (c + 1) * 128], ident[:])
            nc.vector.tensor_copy(v_s[:, c, 64:128], pst[:])

    # ---------------- phase D: attention + O-projection ----------------
    with (
        tc.tile_pool(name="sc_psum", bufs=2, space="PSUM") as sc_psum,
        tc.tile_pool(name="av_psum", bufs=1, space="PSUM") as av_psum,
        tc.tile_pool(name="op_psum", bufs=2, space="PSUM") as op_psum,
        tc.tile_pool(name="pt_pool", bufs=4) as pt_pool,
        tc.tile_pool(name="rrs_pool", bufs=2) as rrs_pool,
        tc.tile_pool(name="ost_pool", bufs=3) as ost_pool,
    ):
        for b in range(BSZ):
            for qb in range(QBLKS):
                qs = b * SEQ + qb * 512
                for hp, (qt, ao) in enumerate(((qt0, ao0), (qt1, ao1))):
                    pse = av_psum.tile([128, 512], F32, name="pse")
                    pso = av_psum.tile([128, 512], F32, name="pso")
                    for kt in range(KT):
                        kr = b * SEQ + kt * 128
                        kc = b * KT + kt
                        psa = sc_psum.tile([128, 1024], F32, name="psa")
                        # scores.T for even head (lanes 0-63) and odd (64-127)
                        nc.tensor.matmul(
                            psa[:, 0:512],
                            k2t[0:64, kr : kr + 128],
                            qt[0:64, qs : qs + 512],
                        )
                        nc.tensor.matmul(
                            psa[:, 512:1024],
                            k2t[64:128, kr : kr + 128],
                            qt[64:128, qs : qs + 512],
                        )
                        pt = pt_pool.tile([128, 1024], BF16, name="pt")
                        nc.scalar.activation(
                            pt[:],
                            psa[:],
                            mybir.ActivationFunctionType.Exp,
                            scale=1.0 / math.sqrt(HEAD_DIM),
                        )
                        st = dict(start=(kt == 0), stop=(kt == KT - 1))
                        # even head: [V|ones] -> rows 0-63 attn, 64-127 rowsums
                        nc.tensor.matmul(
                            pse[:], v_s[:, kc, 64:192], pt[:, 0:512], **st
                        )
                        # odd head: [ones|V] -> rows 0-63 rowsums, 64-127 attn
                        nc.tensor.matmul(
                            pso[:], v_s[:, kc, 0:128], pt[:, 512:1024], **st
                        )
                    # assemble reciprocal rowsums lane-aligned with attn rows
                    rsh = rrs_pool.tile([128, 512], F32, name="rsh")
                    nc.vector.tensor_copy(rsh[64:128, :], pse[64:128, :])
                    nc.vector.tensor_copy(rsh[0:64, :], pso[0:64, :])
                    rrs = rrs_pool.tile([128, 512], F32, name="rrs")
                    nc.sync.dma_start(rrs[0:64, :], rsh[64:128, :])
                    nc.sync.dma_start(rrs[64:128, :], rsh[0:64, :])
                    rri = rrs_pool.tile([128, 512], F32, name="rri")
                    nc.vector.reciprocal_approx_fast(rri[:], rrs[:])
                    nc.vector.tensor_tensor(
                        ao[0:64, qs : qs + 512],
                        pse[0:64, :],
                        rri[0:64, :],
                        mybir.AluOpType.mult,
                    )
                    nc.vector.tensor_tensor(
                        ao[64:128, qs : qs + 512],
                        pso[64:128, :],
                        rri[64:128, :],
                        mybir.AluOpType.mult,
                    )
                # O-projection for these 512 tokens
                for tb in range(4):
                    ts0 = qs + tb * 128
                    for oj in range(4):
                        pop = op_psum.tile([128, 512], F32, name="pop")
                        nc.tensor.matmul(
                            pop[:],
                            ao0[:, ts0 : ts0 + 128],
                            wo_s0[:, oj * 512 : (oj + 1) * 512],
                            start=True,
                            stop=False,
                        )
                        nc.tensor.matmul(
                            pop[:],
                            ao1[:, ts0 : ts0 + 128],
                            wo_s1[:, oj * 512 : (oj + 1) * 512],
                            start=False,
                            stop=True,
                        )
                        ost = ost_pool.tile([128, 512], BF16, name="ost")
                        nc.vector.tensor_copy(ost[:], pop[:])
                        nc.sync.dma_start(
                            out[ts0 : ts0 + 128, oj * 512 : (oj + 1) * 512], ost[:]
                        )


def _host_prep(hidden_states, position_ids, Wq, Wk, Wv, Wo):
    bf = ml_dtypes.bfloat16
    x = np.ascontiguousarray(hidden_states.reshape(T, HIDDEN))
    xT = x.T.astype(bf)  # [HIDDEN, T]
    # block to [NB, HCH, 128, 512] so each projection tile is one contiguous read
    xT = np.ascontiguousarray(
        xT.reshape(HCH, 128, NB // 2, 1024).transpose(2, 0, 1, 3)
    )

    # RoPE tables, transposed to [64, T], sign of sin folded for rotate_half,
    # then stacked twice to cover two heads per SBUF tile.
    inv_freq = 1.0 / (
        ROPE_THETA ** (np.arange(0, HEAD_DIM, 2, dtype=np.float32) / HEAD_DIM)
    )
    pos = position_ids.astype(np.float32).reshape(BSZ, SEQ)
    freqs = pos[:, :, None] * inv_freq[None, None, :]  # [B, S, 32]
    emb = np.concatenate([freqs, freqs], axis=-1)  # [B, S, 64]
    cos = np.cos(emb).reshape(T, HEAD_DIM).T  # [64, T]
    sin = np.sin(emb).reshape(T, HEAD_DIM).T.copy()
    sin[0:32, :] *= -1.0  # rotate_half sign fold
    cosT = np.ascontiguousarray(np.concatenate([cos, cos], axis=0)).astype(bf)
    sinT = np.ascontiguousarray(np.concatenate([sin, sin], axis=0)).astype(bf)

    in_maps = []
    for c in range(NCORES):
        q0 = c * KV_GROUPS * HEAD_DIM  # 256*c
        wq_c = np.ascontiguousarray(Wq[q0 : q0 + 256, :].T).astype(bf)  # [2048, 256]
        wk_c = Wk[c * 64 : (c + 1) * 64, :].T  # [2048, 64]
        wv_c = Wv[c * 64 : (c + 1) * 64, :].T
        wkv_c = np.ascontiguousarray(np.concatenate([wk_c, wv_c], axis=1)).astype(bf)
        wo_c = np.ascontiguousarray(Wo[:, q0 : q0 + 256].T).astype(bf)  # [256, 2048]
        in_maps.append(
            {
                "xT": xT,
                "wq": wq_c,
                "wkv": wkv_c,
                "wo": wo_c,
                "cosT": cosT,
                "sinT": sinT,
            }
        )
    return in_maps


_RUN_KW = {}


def kernel(hidden_states, position_ids, Wq, Wk, Wv, Wo):
    in_maps = _host_prep(hidden_states, position_ids, Wq, Wk, Wv, Wo)
    nc = build_nc()
    res = run_bass_kernel_spmd(nc, in_maps, core_ids=list(range(NCORES)), **_RUN_KW)
    acc = np.zeros((T, HIDDEN), dtype=np.float32)
    for i in range(NCORES):
        acc += res.results[i]["out"].astype(np.float32)
    if _RUN_KW.get("trace"):
        kernel.last_exec_time_ns = res.exec_time_ns
        kernel.last_result = res
    return acc.reshape(BSZ, SEQ, HIDDEN)


# revision 10
# speedup vs baseline: 1.3205x; 1.1327x over previous
"""GQA attention (B=2,S=2048,H=2048, 32 Q heads / 8 KV heads, head_dim 64, RoPE,
full non-causal softmax) on 8 TRN2 NeuronCores.

Sharding: tensor-parallel over KV heads. Core i owns KV head i and Q heads
4i..4i+3 (the GQA group). Each core computes its 4 heads of attention plus the
partial O-projection over its 256 output dims; the 8 partials are summed on the
host (pure unshard of the partial-sum shards).

Device layouts are transposed ("dims on partitions") so every matmul contracts
on the partition axis with zero on-device transposes of activations:
  x.T [2048, 4096]        (host-pretransposed, bf16)
  Q.T [256, 4096] = wqT.T-chunks @ x.T-chunks      (2 SBUF tiles of [128, T])
  K.T/V.T [64, 4096]      (col-tiled into one PSUM bank pair)
  scores.T [keys, q] = K.T-slice.T @ Q.T-slice     (per 128-key tile)
  p.T = exp(scores.T/8)   (ScalarE, PSUM->SBUF bf16, scale fused)
  attn_o.T += V-chunk.T @ p.T-chunk ; rowsums += ones.T @ p.T-chunk
  out[t,:] partial = attn_o.T-chunks.T @ woT-chunks
RoPE rotate_half is a partition swap done with tiny SBUF->SBUF DMAs; the sign
flip is folded into the host-precomputed sin table.
"""

import sys

sys.path.insert(0, "/opt/trn_rl_repo")

import math

import ml_dtypes
import numpy as np

import concourse.bass as bass
import concourse.mybir as mybir
import concourse.tile as tile
from concourse import bacc
from concourse.bass_utils import run_bass_kernel_spmd
from concourse.masks import make_identity

BF16 = mybir.dt.bfloat16
F32 = mybir.dt.float32

HIDDEN = 2048
N_HEADS = 32
N_KV_HEADS = 8
HEAD_DIM = 64
KV_GROUPS = 4
ROPE_THETA = 10000.0
BSZ, SEQ = 2, 2048
T = BSZ * SEQ  # 4096 tokens
HCH = HIDDEN // 128  # 16 hidden chunks
NB = T // 512  # 8 token blocks for projections
KT = SEQ // 128  # 16 key tiles per batch
QBLKS = SEQ // 512  # 4 q blocks of 512 per batch
NCORES = 8


def build_nc(trace_scopes: bool = False):
    nc = bacc.Bacc(None, target_bir_lowering=False, debug=False)

    xT = nc.declare_dram_parameter("xT", [NB // 2, 128, HCH, 1024], BF16, isOutput=False)
    wq = nc.declare_dram_parameter("wq", [HIDDEN, 256], BF16, isOutput=False)
    wkv = nc.declare_dram_parameter("wkv", [HIDDEN, 128], BF16, isOutput=False)
    wo = nc.declare_dram_parameter("wo", [256, HIDDEN], BF16, isOutput=False)
    cosT = nc.declare_dram_parameter("cosT", [128, T], BF16, isOutput=False)
    sinT = nc.declare_dram_parameter("sinT", [128, T], BF16, isOutput=False)
    out = nc.declare_dram_parameter("out", [T, HIDDEN], BF16, isOutput=True)

    with tile.TileContext(nc) as tc:
        _build_body(nc, tc, xT, wq, wkv, wo, cosT, sinT, out)
    nc.compile()
    return nc


def _build_body(nc, tc, xT, wq, wkv, wo, cosT, sinT, out):
    with tc.tile_pool(name="persist", bufs=1) as persist:
        _build_inner(nc, tc, persist, xT, wq, wkv, wo, cosT, sinT, out)


def _build_inner(nc, tc, persist, xT, wq, wkv, wo, cosT, sinT, out):
    # ---------------- persistent SBUF tensors ----------------
    wq_s = persist.tile([128, HCH, 256], BF16, name="wq_s")
    wkv_s = persist.tile([128, HCH, 128], BF16, name="wkv_s")
    wo_s0 = persist.tile([128, HIDDEN], BF16, name="wo_s0")
    wo_s1 = persist.tile([128, HIDDEN], BF16, name="wo_s1")
    cos_s = persist.tile([128, T], BF16, name="cos_s")
    sin_s = persist.tile([128, T], BF16, name="sin_s")
    qt0 = persist.tile([128, T], BF16, name="qt0")  # heads 0(p0-63),1(p64-127)
    qt1 = persist.tile([128, T], BF16, name="qt1")  # heads 2,3
    kvt = persist.tile([128, T], BF16, name="kvt")  # K.T rows 0-63, V.T rows 64-127
    k2t = persist.tile([128, T], BF16, name="k2t")  # K.T duplicated on both halves
    # [ones | V | ones]: even-head lhsT = [:,kc,64:192] = [V|ones],
    # odd-head lhsT = [:,kc,0:128] = [ones|V]
    v_s = persist.tile([128, 2 * KT, 192], BF16, name="v_s")
    ident = persist.tile([64, 64], BF16, name="ident")
    ao0 = persist.tile([128, T], BF16, name="ao0")  # attn_o.T heads 0,1
    ao1 = persist.tile([128, T], BF16, name="ao1")  # attn_o.T heads 2,3

    nc.gpsimd.memset(v_s.rearrange("p c m -> p (c m)")[:, :], 1.0)
    make_identity(nc, ident[:])

    # weight loads
    for c in range(HCH):
        nc.gpsimd.dma_start(wq_s[:, c, :], wq[c * 128 : (c + 1) * 128, :])
        nc.gpsimd.dma_start(wkv_s[:, c, :], wkv[c * 128 : (c + 1) * 128, :])
    nc.gpsimd.dma_start(wo_s0[:], wo[0:128, :])
    nc.gpsimd.dma_start(wo_s1[:], wo[128:256, :])
    nc.gpsimd.dma_start(cos_s[:], cosT[:])
    nc.gpsimd.dma_start(sin_s[:], sinT[:])

    # ---------------- phase B: QKV projections ----------------
    with (
        tc.tile_pool(name="xs_pool", bufs=2) as xs_pool,
        tc.tile_pool(name="proj_psum", bufs=1, space="PSUM") as proj_psum,
    ):
        for np_ in range(NB // 2):
            ts0 = np_ * 1024
            psq0 = proj_psum.tile([128, 1024], F32, name="psq0")
            psq1 = proj_psum.tile([128, 1024], F32, name="psq1")
            pskv = proj_psum.tile([128, 1024], F32, name="pskv")
            xs = xs_pool.tile([128, HCH, 1024], BF16, name="xs")
            nc.sync.dma_start(xs[:], xT[np_])
            for c in range(HCH):
                st = dict(start=(c == 0), stop=(c == HCH - 1))
                for h in range(2):
                    sl = slice(h * 512, (h + 1) * 512)
                    nc.tensor.matmul(psq0[:, sl], wq_s[:, c, 0:128], xs[:, c, sl], **st)
                    nc.tensor.matmul(psq1[:, sl], wq_s[:, c, 128:256], xs[:, c, sl], **st)
                    nc.tensor.matmul(pskv[0:64, sl], wkv_s[:, c, 0:64], xs[:, c, sl], **st)
                    nc.tensor.matmul(pskv[64:128, sl], wkv_s[:, c, 64:128], xs[:, c, sl], **st)
            nc.vector.tensor_copy(qt0[:, ts0 : ts0 + 1024], psq0[:])
            nc.vector.tensor_copy(qt1[:, ts0 : ts0 + 1024], psq1[:])
            nc.vector.tensor_copy(kvt[:, ts0 : ts0 + 1024], pskv[:])

    # ---------------- phase C: RoPE + K duplication + V transpose ------------
    with (
        tc.tile_pool(name="rope_pool", bufs=1) as rope_pool,
        tc.tile_pool(name="tr_psum", bufs=2, space="PSUM") as tr_psum,
        tc.tile_pool(name="vtmp_pool", bufs=1) as vtmp_pool,
    ):
        for qt in (qt0, qt1):
            sq = rope_pool.tile([128, T], BF16, name="sq")
            nc.gpsimd.dma_start(sq[0:32, :], qt[32:64, :])
            nc.gpsimd.dma_start(sq[32:64, :], qt[0:32, :])
            nc.gpsimd.dma_start(sq[64:96, :], qt[96:128, :])
            nc.gpsimd.dma_start(sq[96:128, :], qt[64:96, :])
            nc.vector.tensor_tensor(sq[:], sq[:], sin_s[:], mybir.AluOpType.mult)
            nc.vector.tensor_tensor(qt[:], qt[:], cos_s[:], mybir.AluOpType.mult)
            nc.vector.tensor_tensor(qt[:], qt[:], sq[:], mybir.AluOpType.add)
        # K rows 0-63 of kvt
        sk = rope_pool.tile([64, T], BF16, name="sk")
        nc.gpsimd.dma_start(sk[0:32, :], kvt[32:64, :])
        nc.gpsimd.dma_start(sk[32:64, :], kvt[0:32, :])
        nc.vector.tensor_tensor(sk[:], sk[:], sin_s[0:64, :], mybir.AluOpType.mult)
        nc.vector.tensor_tensor(
            kvt[0:64, :], kvt[0:64, :], cos_s[0:64, :], mybir.AluOpType.mult
        )
        nc.vector.tensor_tensor(kvt[0:64, :], kvt[0:64, :], sk[:], mybir.AluOpType.add)
        # duplicate K.T to both halves of k2t
        nc.gpsimd.dma_start(k2t[0:64, :], kvt[0:64, :])
        nc.gpsimd.dma_start(k2t[64:128, :], kvt[0:64, :])
        # V: move V.T rows 64-127 down to 0-63, then PE-transpose chunks
        vtmp = vtmp_pool.tile([64, T], BF16, name="vtmp")
        nc.gpsimd.dma_start(vtmp[:], kvt[64:128, :])
        for c in range(2 * KT):
            pst = tr_psum.tile([128, 64], BF16, name="pst")
            nc.tensor.transpose(pst[:], vtmp[:, c * 128 : (c + 1) * 128], ident[:])
            nc.vector.tensor_copy(v_s[:, c, 64:128], pst[:])

    # ---------------- phase D: attention + O-projection ----------------
    # O-projection work for block (b,qb) is emitted interleaved into the NEXT
    # block's kt loop so the PE never runs a long oproj burst that starves the
    # exp pipeline on ScalarE.
    with (
        tc.tile_pool(name="sc_psum", bufs=2, space="PSUM") as sc_psum,
        tc.tile_pool(name="av_psum", bufs=1, space="PSUM") as av_psum,
        tc.tile_pool(name="op_psum", bufs=2, space="PSUM") as op_psum,
        tc.tile_pool(name="pt_pool", bufs=4) as pt_pool,
        tc.tile_pool(name="rrs_pool", bufs=2) as rrs_pool,
        tc.tile_pool(name="ost_pool", bufs=2) as ost_pool,
    ):
        pending = []  # list of closures, each emits one oproj unit (2 mm + cast)

        def emit_oproj(qs):
            # 16 units: (tb, oj); one ost staging tile [128, 2048] per tb
            state = {}

            def unit(tb, oj):
                def go():
                    ts0 = qs + tb * 128
                    if oj == 0:
                        state[tb] = ost_pool.tile([128, HIDDEN], BF16, name="ost")
                    ost = state[tb]
                    pop = op_psum.tile([128, 512], F32, name="pop")
                    nc.tensor.matmul(
                        pop[:],
                        ao0[:, ts0 : ts0 + 128],
                        wo_s0[:, oj * 512 : (oj + 1) * 512],
                        start=True,
                        stop=False,
                    )
                    nc.tensor.matmul(
                        pop[:],
                        ao1[:, ts0 : ts0 + 128],
                        wo_s1[:, oj * 512 : (oj + 1) * 512],
                        start=False,
                        stop=True,
                    )
                    nc.vector.tensor_copy(ost[:, oj * 512 : (oj + 1) * 512], pop[:])
                    if oj == 3:
                        nc.sync.dma_start(out[ts0 : ts0 + 128, :], ost[:])

                return go

            return [unit(tb, oj) for tb in range(4) for oj in range(4)]

        for b in range(BSZ):
            for qb in range(QBLKS):
                qs = b * SEQ + qb * 512
                for hp, (qt, ao) in enumerate(((qt0, ao0), (qt1, ao1))):
                    pse = av_psum.tile([128, 512], F32, name="pse")
                    pso = av_psum.tile([128, 512], F32, name="pso")
                    for kt in range(KT):
                        kr = b * SEQ + kt * 128
                        kc = b * KT + kt
                        psa = sc_psum.tile([128, 1024], F32, name="psa")
                        # scores.T for even head (lanes 0-63) and odd (64-127)
                        nc.tensor.matmul(
                            psa[:, 0:512],
                            k2t[0:64, kr : kr + 128],
                            qt[0:64, qs : qs + 512],
                        )
                        nc.tensor.matmul(
                            psa[:, 512:1024],
                            k2t[64:128, kr : kr + 128],
                            qt[64:128, qs : qs + 512],
                        )
                        pt = pt_pool.tile([128, 1024], BF16, name="pt")
                        nc.scalar.activation(
                            pt[:],
                            psa[:],
                            mybir.ActivationFunctionType.Exp,
                            scale=1.0 / math.sqrt(HEAD_DIM),
                        )
                        st = dict(start=(kt == 0), stop=(kt == KT - 1))
                        # even head: [V|ones] -> rows 0-63 attn, 64-127 rowsums
                        nc.tensor.matmul(
                            pse[:], v_s[:, kc, 64:192], pt[:, 0:512], **st
                        )
                        # odd head: [ones|V] -> rows 0-63 rowsums, 64-127 attn
                        nc.tensor.matmul(
                            pso[:], v_s[:, kc, 0:128], pt[:, 512:1024], **st
                        )
                        # drip-feed one pending oproj unit every other kt
                        if kt % 2 == 0 and pending:
                            pending.pop(0)()
                    # assemble reciprocal rowsums lane-aligned with attn rows
                    rsh = rrs_pool.tile([128, 512], F32, name="rsh")
                    nc.vector.tensor_copy(rsh[64:128, :], pse[64:128, :])
                    nc.vector.tensor_copy(rsh[0:64, :], pso[0:64, :])
                    rrs = rrs_pool.tile([128, 512], F32, name="rrs")
                    nc.gpsimd.dma_start(rrs[0:64, :], rsh[64:128, :])
                    nc.gpsimd.dma_start(rrs[64:128, :], rsh[0:64, :])
                    rri = rrs_pool.tile([128, 512], F32, name="rri")
                    nc.vector.reciprocal_approx_fast(rri[:], rrs[:])
                    nc.vector.tensor_tensor(
                        ao[0:64, qs : qs + 512],
                        pse[0:64, :],
                        rri[0:64, :],
                        mybir.AluOpType.mult,
                    )
                    nc.vector.tensor_tensor(
                        ao[64:128, qs : qs + 512],
                        pso[64:128, :],
                        rri[64:128, :],
                        mybir.AluOpType.mult,
                    )
                # flush any leftovers, then queue this block's oproj
                for go in pending:
                    go()
                pending = emit_oproj(qs)
        for go in pending:
            go()


def _host_prep(hidden_states, position_ids, Wq, Wk, Wv, Wo):
    bf = ml_dtypes.bfloat16
    x = np.ascontiguousarray(hidden_states.reshape(T, HIDDEN))
    xT = x.T.astype(bf)  # [HIDDEN, T]
    # block to [NB, HCH, 128, 512] so each projection tile is one contiguous read
    xT = np.ascontiguousarray(
        xT.reshape(HCH, 128, NB // 2, 1024).transpose(2, 1, 0, 3)
    )

    # RoPE tables, transposed to [64, T], sign of sin folded for rotate_half,
    # then stacked twice to cover two heads per SBUF tile.
    inv_freq = 1.0 / (
        ROPE_THETA ** (np.arange(0, HEAD_DIM, 2, dtype=np.float32) / HEAD_DIM)
    )
    pos = position_ids.astype(np.float32).reshape(BSZ, SEQ)
    freqs = pos[:, :, None] * inv_freq[None, None, :]  # [B, S, 32]
    emb = np.concatenate([freqs, freqs], axis=-1)  # [B, S, 64]
    cos = np.cos(emb).reshape(T, HEAD_DIM).T  # [64, T]
    sin = np.sin(emb).reshape(T, HEAD_DIM).T.copy()
    sin[0:32, :] *= -1.0  # rotate_half sign fold
    cosT = np.ascontiguousarray(np.concatenate([cos, cos], axis=0)).astype(bf)
    sinT = np.ascontiguousarray(np.concatenate([sin, sin], axis=0)).astype(bf)

    in_maps = []
    for c in range(NCORES):
        q0 = c * KV_GROUPS * HEAD_DIM  # 256*c
        wq_c = np.ascontiguousarray(Wq[q0 : q0 + 256, :].T).astype(bf)  # [2048, 256]
        wk_c = Wk[c * 64 : (c + 1) * 64, :].T  # [2048, 64]
        wv_c = Wv[c * 64 : (c + 1) * 64, :].T
        wkv_c = np.ascontiguousarray(np.concatenate([wk_c, wv_c], axis=1)).astype(bf)
        wo_c = np.ascontiguousarray(Wo[:, q0 : q0 + 256].T).astype(bf)  # [256, 2048]
        in_maps.append(
            {
                "xT": xT,
                "wq": wq_c,
                "wkv": wkv_c,
                "wo": wo_c,
                "cosT": cosT,
                "sinT": sinT,
            }
        )
    return in_maps


_RUN_KW = {}


def kernel(hidden_states, position_ids, Wq, Wk, Wv, Wo):
    in_maps = _host_prep(hidden_states, position_ids, Wq, Wk, Wv, Wo)
    nc = build_nc()
    res = run_bass_kernel_spmd(nc, in_maps, core_ids=list(range(NCORES)), **_RUN_KW)
    acc = np.zeros((T, HIDDEN), dtype=np.float32)
    for i in range(NCORES):
        acc += res.results[i]["out"].astype(np.float32)
    if _RUN_KW.get("trace"):
        kernel.last_exec_time_ns = res.exec_time_ns
        kernel.last_result = res
    return acc.reshape(BSZ, SEQ, HIDDEN)
